# revision 19
# baseline (speedup 1.0000x reference)
"""MetapathAggrLayer Trainium2 kernel — v4 (hybrid PE/DVE scores, bf16 wsum).

Per node n: e_m = leakyrelu(x[m,n,:].a), w = softmax(e), out = sum_m w_m x[m,n,:].
Data-parallel over N across 8 NeuronCores; nodes-on-partitions layout.

Per 2048-node macro-tile:
  - scores for metapath 0 on TensorE: PE transpose-mode chunks -> PSUM,
    ScalarE copy (fp32->bf16) -> SBUF, tiny matmuls against a host-built
    block-diagonal `a` -> node-major scores in PSUM.
  - scores for metapaths 1-3 on VectorE: fused multiply+prefix-scan custom
    DVE op (v2's), segment sums recovered as prefix differences on GpSimd.
  - exp(leakyrelu(e)) == max(exp(e), exp(0.2*e)): ScalarE Exp x2 + Vector max.
  - weighted sum: broadcast multiplies with stride-0-innermost w
    (metapaths 0+1 as one 2048-col VectorE op, 2+3 on GpSimd), bf16 tree
    adds on VectorE (2x mode), ScalarE converts the bf16 sum to fp32.
Engine budget per ~7-10us macro: V ~10, G ~7, S ~4, PE ~5.5, DMA ~7-10.
"""

import sys

sys.path.insert(0, "/opt/trn_rl_repo")

import ml_dtypes
import numpy as np

import concourse.bacc as bacc
import concourse.mybir as mybir
from concourse import bass_utils, dve_ops, masks
from concourse.dve_spec import Spec, Src0, Src1, scan, AluOp, lower, _has_src1
from concourse.dve_uop import DveOpSpec
from concourse.tile import TileContext

ALPHA = 0.2
NMETA = 4
F = 64
N_FULL = 1_000_000
N_CORES = 8
T = 16                     # t-chunks (nodes per partition) per macro-tile
NODES_PER_MACRO = 128 * T  # 2048
MACROS_PER_CORE = 62
NC_NODES = MACROS_PER_CORE * NODES_PER_MACRO  # 126_976
N_PAD = N_CORES * NC_NODES                    # 1_015_808
TF = T * F                 # 1024 free cols per metapath tile
NCH = TF // 128            # 8 transpose chunks per metapath tile

_CACHE = {}


def _register_op(name, spec, subdim=False):
    if name in dve_ops._SUB_OPCODE_FOR_NAME:
        return next(o for o in dve_ops.OPS if o.name == name)
    row = dve_ops._CUSTOM_DVE_ROW_BASE + len(dve_ops.OPS)
    assert row < 0x20
    shas = {}
    for ver in ("v3", "v4"):
        s = DveOpSpec(name=name, opcode=row, uops=lower(spec, ver=ver),
                      rd1_en=_has_src1(spec))
        shas[ver] = s.sha(ver)
    op = dve_ops.DveOp(name, spec, subdim, shas)
    dve_ops.OPS.append(op)
    dve_ops.CUSTOM_DVE_SPECS[name] = spec
    dve_ops._SUB_OPCODE_FOR_NAME[name] = row
    return op


def _get_scan_mul():
    return _register_op(
        "MPA_SCAN_MUL",
        Spec(
            body=scan(AluOp.ADD, Src0 * Src1),
            reference=lambda in0, in1, *cs: np.cumsum(
                (in0.astype(np.float32) * in1.astype(np.float32)), axis=-1
            ),
        ),
    )


def _build_kernel():
    scan_mul = _get_scan_mul()

    nc = bacc.Bacc("TRN2", target_bir_lowering=False, debug=False)
    f32 = mybir.dt.float32
    bf16 = mybir.dt.bfloat16

    x_in = nc.dram_tensor("input", (NMETA, NC_NODES, F), f32, kind="ExternalInput").ap()
    a2b_in = nc.dram_tensor("a2b", (128, 2), bf16, kind="ExternalInput").ap()
    a_rep_in = nc.dram_tensor("a_rep", (128, TF), f32, kind="ExternalInput").ap()
    out = nc.dram_tensor("out", (NC_NODES, F), f32, kind="ExternalOutput").ap()

    mult = mybir.AluOpType.mult
    add = mybir.AluOpType.add
    subtract = mybir.AluOpType.subtract
    op_max = mybir.AluOpType.max
    Exp = mybir.ActivationFunctionType.Exp

    with TileContext(nc) as tc:
        with tc.tile_pool(name="const", bufs=1) as cpool, \
             tc.tile_pool(name="x", bufs=4) as xpool, \
             tc.tile_pool(name="scan", bufs=3) as scpool, \
             tc.tile_pool(name="wsum", bufs=3) as wpool, \
             tc.tile_pool(name="small", bufs=4) as spool, \
             tc.tile_pool(name="psT", bufs=2, space="PSUM") as ppool, \
             tc.tile_pool(name="psE", bufs=4, space="PSUM") as epool:
            a2b = cpool.tile([128, 2], bf16)
            nc.sync.dma_start(out=a2b[:, :], in_=a2b_in)
            a_rep = cpool.tile([128, TF], f32)
            nc.sync.dma_start(out=a_rep[:, :], in_=a_rep_in)
            ident = cpool.tile([128, 128], f32)
            masks.make_identity(nc, ident[:, :])

            for i in range(MACROS_PER_CORE):
                lo = i * NODES_PER_MACRO

                # ---- load x into one [128, 4096] tile (m-major columns)
                x_all = xpool.tile([128, NMETA * TF], f32, tag="x")
                for m in range(NMETA):
                    for h in range(4):
                        src = x_in[m, lo + h * (TF // 2): lo + (h + 1) * (TF // 2),
                                   :].rearrange("(p t) f -> p (t f)", p=32)
                        nc.sync.dma_start(
                            out=x_all[h * 32:(h + 1) * 32, m * TF:(m + 1) * TF],
                            in_=src)

                e_sb = spool.tile([128, NMETA * T], f32, tag="e")

                # ---- scores m=0 on TensorE (transpose + block-diag matmul)
                xTp = ppool.tile([128, TF], f32, tag="xTp")
                for c in range(NCH):
                    nc.tensor.transpose(
                        xTp[:, c * 128:(c + 1) * 128],
                        x_all[:, c * 128:(c + 1) * 128],
                        ident[:, :],
                    )
                xTs = spool.tile([128, TF], bf16, tag="xTs")
                nc.scalar.copy(out=xTs[:, :], in_=xTp[:, :])
                e_ps = epool.tile([128, T], f32, tag="eps")
                for c in range(NCH):
                    nc.tensor.matmul(
                        e_ps[:, c * 2: c * 2 + 2],
                        lhsT=xTs[:, c * 128:(c + 1) * 128],
                        rhs=a2b[:, :],
                    )
                nc.scalar.copy(out=e_sb[:, 0:T], in_=e_ps[:, :])

                # ---- scores m=1..3 on VectorE (fused mul+scan), diffs on GpSimd
                for m in range(1, NMETA):
                    pm = scpool.tile([128, TF + 1], f32, tag="P")
                    nc.gpsimd.memset(pm[:, 0:1], 0.0)
                    nc.vector._custom_dve(
                        scan_mul, out=pm[:, 1:TF + 1],
                        in0=x_all[:, m * TF:(m + 1) * TF], in1=a_rep[:, :],
                    )
                    p_hi = pm[:, 1:TF + 1].rearrange(
                        "p (t f) -> p t f", f=F)[:, :, F - 1:F]
                    p_lo = pm[:, 0:TF].rearrange(
                        "p (t f) -> p t f", f=F)[:, :, 0:1]
                    nc.gpsimd.tensor_tensor(
                        out=e_sb[:, m * T:(m + 1) * T], in0=p_hi, in1=p_lo,
                        op=subtract)

                # ---- u = exp(leakyrelu(e)) = max(exp(e), exp(alpha*e))
                u1 = spool.tile([128, NMETA * T], f32, tag="u1")
                u2 = spool.tile([128, NMETA * T], f32, tag="u2")
                u = spool.tile([128, NMETA * T], f32, tag="u")
                nc.scalar.activation(u1[:, :], e_sb[:, :], Exp)
                nc.scalar.activation(u2[:, :], e_sb[:, :], Exp, scale=ALPHA)
                nc.vector.tensor_tensor(out=u[:, :], in0=u1[:, :], in1=u2[:, :],
                                        op=op_max)

                # ---- softmax weights: w = u / sum_m u  (cols are m*16 + t)
                s = spool.tile([128, T], f32, tag="s")
                nc.vector.tensor_reduce(
                    out=s[:, :],
                    in_=u[:, :].rearrange("p (m t) -> p t m", m=NMETA),
                    axis=mybir.AxisListType.X,
                    op=add,
                )
                r = spool.tile([128, T], f32, tag="r")
                nc.vector.reciprocal(r[:, :], s[:, :])
                w = spool.tile([128, NMETA * T], bf16, tag="w")
                r_bc = r[:, :].rearrange("p (o t) -> p o t", o=1).broadcast_to(
                    [128, NMETA, T])
                nc.vector.tensor_tensor(
                    out=w[:, :].rearrange("p (m t) -> p m t", m=NMETA),
                    in0=u[:, :].rearrange("p (m t) -> p m t", m=NMETA),
                    in1=r_bc,
                    op=mult,
                )

                # ---- weighted sum: prod = x * w (w broadcast along f, stride-0
                # innermost); m0+m1 as one 2048-col VectorE op, m2+m3 on GpSimd
                prod = wpool.tile([128, NMETA * TF], bf16, tag="prod")
                for half, eng in ((0, nc.vector), (1, nc.gpsimd)):
                    out_v = prod[:, half * 2 * TF:(half + 1) * 2 * TF].rearrange(
                        "p (m t f) -> p m t f", m=2, t=T)
                    in0 = x_all[:, half * 2 * TF:(half + 1) * 2 * TF].rearrange(
                        "p (m t f) -> p m t f", m=2, t=T)
                    in1 = w[:, half * 2 * T:(half + 1) * 2 * T].rearrange(
                        "p (m t o) -> p m t o", m=2, o=1).broadcast_to(
                        [128, 2, T, F])
                    eng.tensor_tensor(out=out_v, in0=in0, in1=in1, op=mult)

                # ---- tree add in bf16 (2x mode), then fp32 convert on ScalarE
                pairsum = wpool.tile([128, 2 * TF], bf16, tag="pair")
                nc.vector.tensor_tensor(
                    out=pairsum[:, :], in0=prod[:, 0:2 * TF],
                    in1=prod[:, 2 * TF:4 * TF], op=add)
                accb = wpool.tile([128, TF], bf16, tag="accb")
                nc.vector.tensor_tensor(
                    out=accb[:, :], in0=pairsum[:, 0:TF], in1=pairsum[:, TF:2 * TF],
                    op=add)
                acc = wpool.tile([128, TF], f32, tag="acc")
                nc.scalar.copy(out=acc[:, :], in_=accb[:, :])

                # ---- store
                for h in range(4):
                    dst = out[lo + h * (TF // 2): lo + (h + 1) * (TF // 2),
                              :].rearrange("(p t) f -> p (t f)", p=32)
                    nc.sync.dma_start(out=dst, in_=acc[h * 32:(h + 1) * 32, :])

    nc.compile()
    return nc


def kernel(input, a, _trace=False):
    input = np.ascontiguousarray(np.asarray(input, dtype=np.float32))
    a = np.asarray(a, dtype=np.float32).reshape(F)

    if "nc" not in _CACHE:
        _CACHE["nc"] = _build_kernel()
    nc = _CACHE["nc"]

    pad = N_PAD - input.shape[1]
    xp = np.concatenate(
        [input, np.zeros((NMETA, pad, F), np.float32)], axis=1
    ) if pad else input

    a2b = np.zeros((128, 2), dtype=ml_dtypes.bfloat16)
    a_bf = a.astype(ml_dtypes.bfloat16)
    a2b[0:64, 0] = a_bf
    a2b[64:128, 1] = a_bf
    a_rep = np.tile(a[None, :], (128, T)).astype(np.float32)

    in_maps = []
    for c in range(N_CORES):
        sl = xp[:, c * NC_NODES:(c + 1) * NC_NODES, :]
        in_maps.append({"input": np.ascontiguousarray(sl), "a2b": a2b,
                        "a_rep": a_rep})

    res = bass_utils.run_bass_kernel_spmd(
        nc, in_maps, core_ids=list(range(N_CORES)), trace=_trace
    )
    outs = [res.results[c]["out"] for c in range(N_CORES)]
    full = np.concatenate(outs, axis=0)[:N_FULL]
    if _trace:
        return full, res
    return full
